# revision 2
# baseline (speedup 1.0000x reference)
"""Trainium2 Bass kernel for DYSPN-style dynamic local filtering.

Computation (per batch b, pixel p):
    patches[j,p] = 7x7 im2col of `input` (zero pad 3), center tap replaced by input0
    scale[j,p]   = attention[b, i, ring(j), p]      (ring in {0..3}, scale >= 0)
    w            = kernel * scale;  w /= sum_j |w|
    out[p]       = sum_j patches[j,p] * w[j,p]

Since scale >= 0 and constant within a ring:
    out = (sum_r att_r * B_r) / (sum_r att_r * A_r)
    B_r = sum_{j in ring r} patches_j * k_j,   A_r = sum_{j in ring r} |k_j|

Sharding: 8 cores = 4 batches x 2 half-images (128 rows each). Per core the
output plane is [128 rows (partitions), 320 cols (free)]; tap shifts become
free-dim offsets into 7 pre-shifted padded-image variants (host-built), so
every tap multiply is a full-partition [128,320] op. Kernel taps are
ring-reordered on host so each ring reduces with one strided tensor_reduce.
"""

import sys

for _p in ("/opt/trn_rl_repo", "/root/.axon_site"):
    if _p not in sys.path:
        sys.path.insert(0, _p)

import numpy as np
from contextlib import ExitStack

import concourse.bass as bass
import concourse.tile as tile
from concourse import bacc, mybir
from concourse.bass_utils import run_bass_kernel_spmd

H, W = 256, 320
BS = 4
KK = 49
HALF_ROWS = 128
HW_HALF = HALF_ROWS * W  # 40960
PAD_W = W + 6  # 326

def _ring_ids() -> np.ndarray:
    line7 = [j for j in range(KK) if 0 <= j <= 7 or 13 <= j <= 14 or 20 <= j <= 21
             or 27 <= j <= 28 or 34 <= j <= 35 or 41 <= j <= 48]
    line5 = [j for j in range(KK) if 8 <= j <= 12 or j in (15, 19, 22, 26, 29, 33)
             or 36 <= j <= 40]
    line3 = [j for j in range(KK) if 16 <= j <= 18 or j in (23, 25) or 30 <= j <= 32]
    ring = np.zeros(KK, dtype=np.int32)
    ring[line7] = 3
    ring[line5] = 2
    ring[line3] = 1
    ring[24] = 0
    return ring

_RING = _ring_ids()
# taps grouped by ring: ring0 (center, 1), ring1 (8), ring2 (16), ring3 (24)
RING_ORDER = np.concatenate([np.where(_RING == r)[0] for r in range(4)]).astype(np.int64)
RING_SIZES = [int((_RING == r).sum()) for r in range(4)]  # [1, 8, 16, 24]
RING_OFFS = np.concatenate([[0], np.cumsum(RING_SIZES)]).tolist()  # [0,1,9,25,49]

_NC = None
LAST_RESULTS = None


def _build_program():
    f32 = mybir.dt.float32
    nc = bacc.Bacc("TRN2", target_bir_lowering=False, debug=False, num_devices=8)
    k_d = nc.dram_tensor("k", [HALF_ROWS, KK, W], f32, kind="ExternalInput").ap()
    img7_d = nc.dram_tensor("img7", [HALF_ROWS, 7, PAD_W], f32, kind="ExternalInput").ap()
    in0_d = nc.dram_tensor("in0", [HALF_ROWS, W], f32, kind="ExternalInput").ap()
    att_d = nc.dram_tensor("att", [HALF_ROWS, 4, W], f32, kind="ExternalInput").ap()
    out_d = nc.dram_tensor("out", [HALF_ROWS, W], f32, kind="ExternalOutput").ap()

    with tile.TileContext(nc) as tc, ExitStack() as ctx:
        pool = ctx.enter_context(tc.tile_pool(name="main", bufs=1))

        img7_t = pool.tile([HALF_ROWS, 7, PAD_W], f32)
        nc.sync.dma_start(img7_t[:], img7_d[:])
        in0_t = pool.tile([HALF_ROWS, W], f32)
        nc.sync.dma_start(in0_t[:], in0_d[:])

        k_tiles = []
        for g in range(4):
            sz = RING_SIZES[g]
            kt = pool.tile([HALF_ROWS, sz, W], f32, name=f"k_ring{g}")
            nc.sync.dma_start(kt[:], k_d[:, RING_OFFS[g]:RING_OFFS[g + 1], :])
            k_tiles.append(kt)

        att_t = pool.tile([HALF_ROWS, 4, W], f32)
        nc.sync.dma_start(att_t[:], att_d[:])

        a_t = pool.tile([HALF_ROWS, 4, W], f32)  # A_r: ring abs-sums of k
        b_t = pool.tile([HALF_ROWS, 4, W], f32)  # B_r: ring sums of patches*k

        for g in range(4):
            sz = RING_SIZES[g]
            kt = k_tiles[g]
            # A_r = sum_j |k_j|  (strided reduce over the tap axis)
            nc.vector.tensor_reduce(
                a_t[:, g, :], kt[:].rearrange("p j x -> p x j"),
                axis=mybir.AxisListType.X, op=mybir.AluOpType.add,
                apply_absolute_value=True,
            )
            # k_j *= patches_j  (in place; center tap uses input0)
            for jj in range(sz):
                j_orig = int(RING_ORDER[RING_OFFS[g] + jj])
                dy, dx = divmod(j_orig, 7)
                if g == 0:
                    src = in0_t[:, :]
                else:
                    src = img7_t[:, dy, dx:dx + W]
                nc.vector.tensor_mul(kt[:, jj, :], kt[:, jj, :], src)
            # B_r = sum_j patches_j * k_j
            nc.vector.tensor_reduce(
                b_t[:, g, :], kt[:].rearrange("p j x -> p x j"),
                axis=mybir.AxisListType.X, op=mybir.AluOpType.add,
            )

        pd_t = pool.tile([HALF_ROWS, 4, W], f32)
        nc.vector.tensor_mul(pd_t[:], att_t[:], a_t[:])
        den_t = pool.tile([HALF_ROWS, W], f32)
        nc.vector.tensor_reduce(
            den_t[:], pd_t[:].rearrange("p r x -> p x r"),
            axis=mybir.AxisListType.X, op=mybir.AluOpType.add,
        )
        pn_t = pool.tile([HALF_ROWS, 4, W], f32)
        nc.vector.tensor_mul(pn_t[:], att_t[:], b_t[:])
        num_t = pool.tile([HALF_ROWS, W], f32)
        nc.vector.tensor_reduce(
            num_t[:], pn_t[:].rearrange("p r x -> p x r"),
            axis=mybir.AxisListType.X, op=mybir.AluOpType.add,
        )
        rden_t = pool.tile([HALF_ROWS, W], f32)
        nc.vector.reciprocal(rden_t[:], den_t[:])
        out_t = pool.tile([HALF_ROWS, W], f32)
        nc.vector.tensor_mul(out_t[:], num_t[:], rden_t[:])
        nc.sync.dma_start(out_d[:], out_t[:])

    nc.compile()
    return nc


def _get_program():
    global _NC
    if _NC is None:
        _NC = _build_program()
    return _NC


def kernel(**inputs) -> np.ndarray:
    k = np.asarray(inputs["kernel"], dtype=np.float32)      # [4, 49, 81920]
    img = np.asarray(inputs["input"], dtype=np.float32)     # [4, 1, 256, 320]
    in0 = np.asarray(inputs["input0"], dtype=np.float32)    # [4, 1, 256, 320]
    att = np.asarray(inputs["attention"], dtype=np.float32) # [4, 6, 4, 81920]
    ii = int(np.asarray(inputs["i"]))

    nc = _get_program()

    in_maps = []
    for c in range(8):
        b, half = divmod(c, 2)
        y0 = half * HALF_ROWS
        kc = k[b][RING_ORDER][:, y0 * W:(y0 + HALF_ROWS) * W]
        kc = np.ascontiguousarray(kc.reshape(KK, HALF_ROWS, W).transpose(1, 0, 2))
        pad = np.zeros((HALF_ROWS + 6, PAD_W), np.float32)
        lo, hi = max(0, y0 - 3), min(H, y0 + HALF_ROWS + 3)
        pad[lo - (y0 - 3):hi - (y0 - 3), 3:3 + W] = img[b, 0, lo:hi]
        img7 = np.ascontiguousarray(
            np.stack([pad[t:t + HALF_ROWS] for t in range(7)], axis=1))
        in0c = np.ascontiguousarray(in0[b, 0, y0:y0 + HALF_ROWS])
        attc = att[b, ii][:, y0 * W:(y0 + HALF_ROWS) * W]
        attc = np.ascontiguousarray(attc.reshape(4, HALF_ROWS, W).transpose(1, 0, 2))
        in_maps.append({"k": kc, "img7": img7, "in0": in0c, "att": attc})

    res = run_bass_kernel_spmd(nc, in_maps, list(range(8)))
    global LAST_RESULTS
    LAST_RESULTS = res

    out = np.empty((BS, 1, H, W), np.float32)
    for c in range(8):
        b, half = divmod(c, 2)
        out[b, 0, half * HALF_ROWS:(half + 1) * HALF_ROWS] = res.results[c]["out"]
    return out


# revision 4
# speedup vs baseline: 1.0702x; 1.0702x over previous
"""Trainium2 Bass kernel for DYSPN-style dynamic local filtering.

Computation (per batch b, pixel p):
    patches[j,p] = 7x7 im2col of `input` (zero pad 3), center tap replaced by input0
    scale[j,p]   = attention[b, i, ring(j), p]      (ring in {0..3}, scale >= 0)
    w            = kernel * scale;  w /= sum_j |w|
    out[p]       = sum_j patches[j,p] * w[j,p]

Since scale >= 0 and constant within a ring (ring = Chebyshev distance from
the center tap):
    out = (sum_r att_r * B_r) / (sum_r att_r * A_r)
    B_r = sum_{j in ring r} patches_j * k_j,   A_r = sum_{j in ring r} |k_j|

Sharding: 8 cores = 4 batches x 2 half-images (128 rows each). Per core the
output plane is [128 rows (partitions), 320 cols (free)]; tap shifts become
free-dim offsets into 7 pre-shifted padded-image variants (host-built), so
every tap multiply is a full-partition [128,320] op (multiple taps of one
image row fuse into a single op via an overlapping strided view).

Engine split per core:
  ACT:  |k| into kabs tiles (one op per 8-plane group)
  DVE:  tap multiplies (in place into k tiles) + pairwise-add trees for B_r
        + attention combine + reciprocal
  POOL: pairwise-add trees for A_r (runs concurrently with DVE: fp32
        tensor_tensor never uses the shared SBUF port pair)
Taps are ring-reordered on host so each ring is a contiguous group of planes.
"""

import sys

for _p in ("/opt/trn_rl_repo", "/root/.axon_site"):
    if _p not in sys.path:
        sys.path.insert(0, _p)

import numpy as np
from contextlib import ExitStack

import concourse.bass as bass
import concourse.tile as tile
from concourse import bacc, mybir
from concourse.bass_utils import run_bass_kernel_spmd

H, W = 256, 320
BS = 4
KK = 49
HALF_ROWS = 128
PAD_W = W + 6  # 326

def _ring_ids() -> np.ndarray:
    ring = np.zeros(KK, dtype=np.int32)
    for j in range(KK):
        dy, dx = divmod(j, 7)
        ring[j] = max(abs(dy - 3), abs(dx - 3))
    return ring

_RING = _ring_ids()
RING_TAPS = [np.where(_RING == r)[0].tolist() for r in range(4)]  # sizes 1,8,16,24
RING_ORDER = np.concatenate([np.asarray(t) for t in RING_TAPS]).astype(np.int64)

# groups: (ring, taps) — rings 2/3 split into 8-plane chunks for pipelining
GROUPS = []
for _r in range(4):
    t = RING_TAPS[_r]
    for _o in range(0, len(t), 8):
        GROUPS.append((_r, t[_o:_o + 8]))
# ring -> index of the group holding that ring's tree result (first group)
RING_HEAD_GROUP = {0: 0, 1: 1, 2: 2, 3: 4}


def _runs(taps):
    """Maximal runs of taps with the same dy and a uniform dx step.

    Returns (pos, dy, dx0, step, length) — such a run multiplies with one
    strided view of the shifted-image tile.
    """
    runs = []
    i = 0
    while i < len(taps):
        dy0, dx0 = divmod(taps[i], 7)
        L, step = 1, 1
        if i + 1 < len(taps):
            dy1, dx1 = divmod(taps[i + 1], 7)
            if dy1 == dy0 and dx1 > dx0:
                step = dx1 - dx0
                while (i + L < len(taps)
                       and divmod(taps[i + L], 7) == (dy0, dx0 + L * step)):
                    L += 1
        runs.append((i, dy0, dx0, step, L))
        i += L
    return runs


_NC = None
LAST_RESULTS = None


def _tree_sum(eng, kt, sz):
    """Pairwise in-place sum of planes [0, sz) of kt into plane 0."""
    cur = sz
    while cur > 1:
        h = cur // 2
        eng.tensor_add(kt[:, 0:h, :], kt[:, 0:h, :], kt[:, h:2 * h, :])
        if cur % 2:
            eng.tensor_add(kt[:, 0, :], kt[:, 0, :], kt[:, cur - 1, :])
        cur = h


def _build_program():
    f32 = mybir.dt.float32
    nc = bacc.Bacc("TRN2", target_bir_lowering=False, debug=False, num_devices=8)
    k_d = nc.dram_tensor("k", [HALF_ROWS, KK, W], f32, kind="ExternalInput").ap()
    img7_d = nc.dram_tensor("img7", [HALF_ROWS, 7, PAD_W], f32, kind="ExternalInput").ap()
    in0_d = nc.dram_tensor("in0", [HALF_ROWS, W], f32, kind="ExternalInput").ap()
    att_d = nc.dram_tensor("att", [HALF_ROWS, 4, W], f32, kind="ExternalInput").ap()
    out_d = nc.dram_tensor("out", [HALF_ROWS, W], f32, kind="ExternalOutput").ap()

    with tile.TileContext(nc) as tc, ExitStack() as ctx:
        pool = ctx.enter_context(tc.tile_pool(name="main", bufs=1))

        img7_t = pool.tile([HALF_ROWS, 7, PAD_W], f32)
        nc.sync.dma_start(img7_t[:], img7_d[:])
        in0_t = pool.tile([HALF_ROWS, W], f32)
        nc.sync.dma_start(in0_t[:], in0_d[:])

        k_tiles = []
        off = 0
        for gi, (r, taps) in enumerate(GROUPS):
            kt = pool.tile([HALF_ROWS, len(taps), W], f32, name=f"k_g{gi}")
            nc.sync.dma_start(kt[:], k_d[:, off:off + len(taps), :])
            k_tiles.append(kt)
            off += len(taps)

        att_t = pool.tile([HALF_ROWS, 4, W], f32)
        nc.sync.dma_start(att_t[:], att_d[:])

        img7_ap = img7_t[:]
        part_dim = img7_ap.ap[0]  # [stride, 128]

        kabs_tiles = []
        for gi, (r, taps) in enumerate(GROUPS):
            kt = k_tiles[gi]
            sz = len(taps)
            # |k| on ScalarE (must read k before the in-place multiplies)
            ka = pool.tile([HALF_ROWS, sz, W], f32, name=f"kabs_g{gi}")
            nc.scalar.activation(ka[:], kt[:], mybir.ActivationFunctionType.Abs)
            kabs_tiles.append(ka)
            # k_j *= patches_j on DVE (fused over same-dy runs)
            if r == 0:
                nc.vector.tensor_mul(kt[:, 0, :], kt[:, 0, :], in0_t[:])
            else:
                for (pos, dy, dx0, step, L) in _runs(taps):
                    if L == 1:
                        src = img7_ap[:, dy, dx0:dx0 + W]
                        nc.vector.tensor_mul(kt[:, pos, :], kt[:, pos, :], src)
                    else:
                        src = bass.AP(
                            img7_ap.tensor,
                            img7_ap.offset + dy * PAD_W + dx0,
                            [part_dim, [step, L], [1, W]],
                        )
                        nc.vector.tensor_mul(kt[:, pos:pos + L, :],
                                             kt[:, pos:pos + L, :], src)
            # B subtree on DVE, A subtree on POOL
            if sz > 1:
                _tree_sum(nc.vector, kt, sz)
                _tree_sum(nc.gpsimd, ka, sz)

        # merge sub-group results: ring2 = g2+g3, ring3 = g4+g5+g6
        nc.vector.tensor_add(k_tiles[2][:, 0, :], k_tiles[2][:, 0, :], k_tiles[3][:, 0, :])
        nc.vector.tensor_add(k_tiles[4][:, 0, :], k_tiles[4][:, 0, :], k_tiles[5][:, 0, :])
        nc.vector.tensor_add(k_tiles[4][:, 0, :], k_tiles[4][:, 0, :], k_tiles[6][:, 0, :])
        nc.gpsimd.tensor_add(kabs_tiles[2][:, 0, :], kabs_tiles[2][:, 0, :], kabs_tiles[3][:, 0, :])
        nc.gpsimd.tensor_add(kabs_tiles[4][:, 0, :], kabs_tiles[4][:, 0, :], kabs_tiles[5][:, 0, :])
        nc.gpsimd.tensor_add(kabs_tiles[4][:, 0, :], kabs_tiles[4][:, 0, :], kabs_tiles[6][:, 0, :])

        # combine: numer = sum_r att_r*B_r, denom = sum_r att_r*A_r
        pn_t = pool.tile([HALF_ROWS, 4, W], f32)
        pd_t = pool.tile([HALF_ROWS, 4, W], f32)
        for r in range(4):
            g = RING_HEAD_GROUP[r]
            nc.vector.tensor_mul(pn_t[:, r, :], att_t[:, r, :], k_tiles[g][:, 0, :])
            nc.vector.tensor_mul(pd_t[:, r, :], att_t[:, r, :], kabs_tiles[g][:, 0, :])
        _tree_sum(nc.vector, pn_t, 4)
        _tree_sum(nc.vector, pd_t, 4)

        rden_t = pool.tile([HALF_ROWS, W], f32)
        nc.vector.reciprocal(rden_t[:], pd_t[:, 0, :])
        out_t = pool.tile([HALF_ROWS, W], f32)
        nc.vector.tensor_mul(out_t[:], pn_t[:, 0, :], rden_t[:])
        nc.sync.dma_start(out_d[:], out_t[:])

    nc.compile()
    return nc


def _get_program():
    global _NC
    if _NC is None:
        _NC = _build_program()
    return _NC


def kernel(**inputs) -> np.ndarray:
    k = np.asarray(inputs["kernel"], dtype=np.float32)      # [4, 49, 81920]
    img = np.asarray(inputs["input"], dtype=np.float32)     # [4, 1, 256, 320]
    in0 = np.asarray(inputs["input0"], dtype=np.float32)    # [4, 1, 256, 320]
    att = np.asarray(inputs["attention"], dtype=np.float32) # [4, 6, 4, 81920]
    ii = int(np.asarray(inputs["i"]))

    nc = _get_program()

    in_maps = []
    for c in range(8):
        b, half = divmod(c, 2)
        y0 = half * HALF_ROWS
        kc = k[b][RING_ORDER][:, y0 * W:(y0 + HALF_ROWS) * W]
        kc = np.ascontiguousarray(kc.reshape(KK, HALF_ROWS, W).transpose(1, 0, 2))
        pad = np.zeros((HALF_ROWS + 6, PAD_W), np.float32)
        lo, hi = max(0, y0 - 3), min(H, y0 + HALF_ROWS + 3)
        pad[lo - (y0 - 3):hi - (y0 - 3), 3:3 + W] = img[b, 0, lo:hi]
        img7 = np.ascontiguousarray(
            np.stack([pad[t:t + HALF_ROWS] for t in range(7)], axis=1))
        in0c = np.ascontiguousarray(in0[b, 0, y0:y0 + HALF_ROWS])
        attc = att[b, ii][:, y0 * W:(y0 + HALF_ROWS) * W]
        attc = np.ascontiguousarray(attc.reshape(4, HALF_ROWS, W).transpose(1, 0, 2))
        in_maps.append({"k": kc, "img7": img7, "in0": in0c, "att": attc})

    res = run_bass_kernel_spmd(nc, in_maps, list(range(8)))
    global LAST_RESULTS
    LAST_RESULTS = res

    out = np.empty((BS, 1, H, W), np.float32)
    for c in range(8):
        b, half = divmod(c, 2)
        out[b, 0, half * HALF_ROWS:(half + 1) * HALF_ROWS] = res.results[c]["out"]
    return out


# revision 8
# speedup vs baseline: 1.2521x; 1.1700x over previous
"""Trainium2 Bass kernel for DYSPN-style dynamic local filtering.

Computation (per batch b, pixel p):
    patches[j,p] = 7x7 im2col of `input` (zero pad 3), center tap replaced by input0
    scale[j,p]   = attention[b, i, ring(j), p]      (ring in {0..3}, scale >= 0)
    w            = kernel * scale;  w /= sum_j |w|
    out[p]       = sum_j patches[j,p] * w[j,p]

Since scale >= 0 and constant within a ring (ring = Chebyshev distance from
the center tap):
    out = (sum_r att_r * B_r) / (sum_r att_r * A_r)
    B_r = sum_{j in ring r} patches_j * k_j,   A_r = sum_{j in ring r} |k_j|

Sharding: 8 cores = 4 batches x 2 half-images (128 rows each). Per core the
output plane is [128 rows (partitions), 320 cols (free)]; tap shifts become
free-dim offsets into 7 pre-shifted padded-image variants (host-built), so
every tap multiply is a full-partition [128,320] op (multiple taps of one
image row fuse into a single op via an overlapping strided view).

Engine split per core:
  ACT:  |k| into kabs tiles (one op per 8-plane group)
  DVE:  tap multiplies (in place into k tiles) + pairwise-add trees for B_r
        + attention combine + reciprocal
  POOL: pairwise-add trees for A_r (runs concurrently with DVE: fp32
        tensor_tensor never uses the shared SBUF port pair)
Taps are ring-reordered on host so each ring is a contiguous group of planes.
"""

import sys

for _p in ("/opt/trn_rl_repo", "/root/.axon_site"):
    if _p not in sys.path:
        sys.path.insert(0, _p)

import numpy as np
from contextlib import ExitStack

import concourse.bass as bass
import concourse.tile as tile
from concourse import bacc, mybir
from concourse.bass_utils import run_bass_kernel_spmd

H, W = 256, 320
BS = 4
KK = 49
HALF_ROWS = 128
PAD_W = W + 6  # 326

def _ring_ids() -> np.ndarray:
    ring = np.zeros(KK, dtype=np.int32)
    for j in range(KK):
        dy, dx = divmod(j, 7)
        ring[j] = max(abs(dy - 3), abs(dx - 3))
    return ring

_RING = _ring_ids()
RING_TAPS = [np.where(_RING == r)[0].tolist() for r in range(4)]  # sizes 1,8,16,24
RING_ORDER = np.concatenate([np.asarray(t) for t in RING_TAPS]).astype(np.int64)

# groups: (ring, taps) — rings 2/3 split into 8-plane chunks for pipelining
GROUPS = []
for _r in range(4):
    t = RING_TAPS[_r]
    for _o in range(0, len(t), 8):
        GROUPS.append((_r, t[_o:_o + 8]))
# ring -> index of the group holding that ring's tree result (first group)
RING_HEAD_GROUP = {0: 0, 1: 1, 2: 2, 3: 4}


def _runs(taps):
    """Maximal runs of taps with the same dy and a uniform dx step.

    Returns (pos, dy, dx0, step, length) — such a run multiplies with one
    strided view of the shifted-image tile.
    """
    runs = []
    i = 0
    while i < len(taps):
        dy0, dx0 = divmod(taps[i], 7)
        L, step = 1, 1
        if i + 1 < len(taps):
            dy1, dx1 = divmod(taps[i + 1], 7)
            if dy1 == dy0 and dx1 > dx0:
                step = dx1 - dx0
                while (i + L < len(taps)
                       and divmod(taps[i + L], 7) == (dy0, dx0 + L * step)):
                    L += 1
        runs.append((i, dy0, dx0, step, L))
        i += L
    return runs


_NC = None
LAST_RESULTS = None


def _tree_sum(eng, kt, sz, dst=None):
    """Pairwise in-place sum of planes [0, sz) of kt into plane 0.

    sz must be a power of two. If dst is given, the final 2->1 level
    writes there instead.
    """
    assert sz & (sz - 1) == 0
    cur = sz
    while cur > 1:
        h = cur // 2
        out = dst if (dst is not None and cur == 2) else kt[:, 0:h, :]
        eng.tensor_add(out, kt[:, 0:h, :], kt[:, h:2 * h, :])
        cur = h


def _build_program():
    f32 = mybir.dt.float32
    nc = bacc.Bacc("TRN2", target_bir_lowering=False, debug=False, num_devices=8)
    k_d = nc.dram_tensor("k", [HALF_ROWS, KK, W], f32, kind="ExternalInput").ap()
    img7_d = nc.dram_tensor("img7", [HALF_ROWS, 7, PAD_W], f32, kind="ExternalInput").ap()
    in0_d = nc.dram_tensor("in0", [HALF_ROWS, W], f32, kind="ExternalInput").ap()
    att_d = nc.dram_tensor("att", [HALF_ROWS, 4, W], f32, kind="ExternalInput").ap()
    out_d = nc.dram_tensor("out", [HALF_ROWS, W], f32, kind="ExternalOutput").ap()

    with tile.TileContext(nc) as tc, ExitStack() as ctx:
        pool = ctx.enter_context(tc.tile_pool(name="main", bufs=1))

        in0_t = pool.tile([HALF_ROWS, W], f32)
        nc.sync.dma_start(in0_t[:], in0_d[:])
        img7_t = pool.tile([HALF_ROWS, 7, PAD_W], f32)
        nc.sync.dma_start(img7_t[:], img7_d[:])

        k_tiles = []
        off = 0
        for gi, (r, taps) in enumerate(GROUPS):
            kt = pool.tile([HALF_ROWS, len(taps), W], f32, name=f"k_g{gi}")
            nc.sync.dma_start(kt[:], k_d[:, off:off + len(taps), :])
            k_tiles.append(kt)
            off += len(taps)

        att_t = pool.tile([HALF_ROWS, 4, W], f32)
        nc.sync.dma_start(att_t[:], att_d[:])

        img7_ap = img7_t[:]
        part_dim = img7_ap.ap[0]  # [stride, 128]

        res_b = pool.tile([HALF_ROWS, 4, W], f32)  # B_r per ring
        res_a = pool.tile([HALF_ROWS, 4, W], f32)  # A_r per ring

        def _mul_group(gi, taps, r):
            """k_j *= patches_j, fused over same-dy/uniform-step runs."""
            kt = k_tiles[gi]
            for (pos, dy, dx0, step, L) in _runs(taps):
                if L == 1:
                    src = img7_ap[:, dy, dx0:dx0 + W]
                    nc.vector.tensor_mul(kt[:, pos, :], kt[:, pos, :], src)
                else:
                    src = bass.AP(
                        img7_ap.tensor,
                        img7_ap.offset + dy * PAD_W + dx0,
                        [part_dim, [step, L], [1, W]],
                    )
                    nc.vector.tensor_mul(kt[:, pos:pos + L, :],
                                         kt[:, pos:pos + L, :], src)

        kabs_tiles = {}
        for gi, (r, taps) in enumerate(GROUPS):
            kt = k_tiles[gi]
            sz = len(taps)
            # |k| on ScalarE (reads k before the in-place multiplies below)
            if gi == 0:
                nc.scalar.activation(res_a[:, 0, :], kt[:, 0, :],
                                     mybir.ActivationFunctionType.Abs)
            else:
                ka = pool.tile([HALF_ROWS, sz, W], f32, name=f"kabs_g{gi}")
                nc.scalar.activation(ka[:], kt[:], mybir.ActivationFunctionType.Abs)
                kabs_tiles[gi] = ka
            if gi == 0:
                nc.vector.tensor_mul(res_b[:, 0, :], kt[:, 0, :], in0_t[:])
            else:
                _mul_group(gi, taps, r)
                # B subtree; ring1 (single group) lands directly in res_b
                bdst = res_b[:, 1, :] if gi == 1 else None
                adst = res_a[:, 1, :] if gi == 1 else None
                _tree_sum(nc.vector, kt, sz, dst=bdst)
                _tree_sum(nc.vector, kabs_tiles[gi], sz, dst=adst)

        # ring results: ring2 = g2+g3; ring3 = g4+g5+g6
        nc.vector.tensor_add(res_b[:, 2, :], k_tiles[2][:, 0, :], k_tiles[3][:, 0, :])
        nc.vector.tensor_add(k_tiles[4][:, 0, :], k_tiles[4][:, 0, :], k_tiles[5][:, 0, :])
        nc.vector.tensor_add(res_b[:, 3, :], k_tiles[4][:, 0, :], k_tiles[6][:, 0, :])
        nc.vector.tensor_add(res_a[:, 2, :], kabs_tiles[2][:, 0, :], kabs_tiles[3][:, 0, :])
        nc.vector.tensor_add(kabs_tiles[4][:, 0, :], kabs_tiles[4][:, 0, :], kabs_tiles[5][:, 0, :])
        nc.vector.tensor_add(res_a[:, 3, :], kabs_tiles[4][:, 0, :], kabs_tiles[6][:, 0, :])

        # combine: numer = sum_r att_r*B_r, denom = sum_r att_r*A_r
        pn_t = pool.tile([HALF_ROWS, 4, W], f32)
        nc.vector.tensor_mul(pn_t[:], att_t[:], res_b[:])
        pd_t = pool.tile([HALF_ROWS, 4, W], f32)
        nc.vector.tensor_mul(pd_t[:], att_t[:], res_a[:])
        _tree_sum(nc.vector, pn_t, 4)
        _tree_sum(nc.vector, pd_t, 4)

        rden_t = pool.tile([HALF_ROWS, W], f32)
        nc.vector.reciprocal(rden_t[:], pd_t[:, 0, :])
        out_t = pool.tile([HALF_ROWS, W], f32)
        nc.vector.tensor_mul(out_t[:], pn_t[:, 0, :], rden_t[:])
        nc.sync.dma_start(out_d[:], out_t[:])

    nc.compile()
    return nc


def _get_program():
    global _NC
    if _NC is None:
        _NC = _build_program()
    return _NC


def kernel(**inputs) -> np.ndarray:
    k = np.asarray(inputs["kernel"], dtype=np.float32)      # [4, 49, 81920]
    img = np.asarray(inputs["input"], dtype=np.float32)     # [4, 1, 256, 320]
    in0 = np.asarray(inputs["input0"], dtype=np.float32)    # [4, 1, 256, 320]
    att = np.asarray(inputs["attention"], dtype=np.float32) # [4, 6, 4, 81920]
    ii = int(np.asarray(inputs["i"]))

    nc = _get_program()

    in_maps = []
    for c in range(8):
        b, half = divmod(c, 2)
        y0 = half * HALF_ROWS
        kc = k[b][RING_ORDER][:, y0 * W:(y0 + HALF_ROWS) * W]
        kc = np.ascontiguousarray(kc.reshape(KK, HALF_ROWS, W).transpose(1, 0, 2))
        pad = np.zeros((HALF_ROWS + 6, PAD_W), np.float32)
        lo, hi = max(0, y0 - 3), min(H, y0 + HALF_ROWS + 3)
        pad[lo - (y0 - 3):hi - (y0 - 3), 3:3 + W] = img[b, 0, lo:hi]
        img7 = np.ascontiguousarray(
            np.stack([pad[t:t + HALF_ROWS] for t in range(7)], axis=1))
        in0c = np.ascontiguousarray(in0[b, 0, y0:y0 + HALF_ROWS])
        attc = att[b, ii][:, y0 * W:(y0 + HALF_ROWS) * W]
        attc = np.ascontiguousarray(attc.reshape(4, HALF_ROWS, W).transpose(1, 0, 2))
        in_maps.append({"k": kc, "img7": img7, "in0": in0c, "att": attc})

    res = run_bass_kernel_spmd(nc, in_maps, list(range(8)))
    global LAST_RESULTS
    LAST_RESULTS = res

    out = np.empty((BS, 1, H, W), np.float32)
    for c in range(8):
        b, half = divmod(c, 2)
        out[b, 0, half * HALF_ROWS:(half + 1) * HALF_ROWS] = res.results[c]["out"]
    return out


# revision 9
# speedup vs baseline: 1.2833x; 1.0249x over previous
"""Trainium2 Bass kernel for DYSPN-style dynamic local filtering.

Computation (per batch b, pixel p):
    patches[j,p] = 7x7 im2col of `input` (zero pad 3), center tap replaced by input0
    scale[j,p]   = attention[b, i, ring(j), p]      (ring in {0..3}, scale >= 0)
    w            = kernel * scale;  w /= sum_j |w|
    out[p]       = sum_j patches[j,p] * w[j,p]

Since scale >= 0 and constant within a ring (ring = Chebyshev distance from
the center tap):
    out = (sum_r att_r * B_r) / (sum_r att_r * A_r)
    B_r = sum_{j in ring r} patches_j * k_j,   A_r = sum_{j in ring r} |k_j|

Sharding: 8 cores = 4 batches x 2 half-images (128 rows each). Per core the
output plane is [128 rows (partitions), 320 cols (free)]; tap shifts become
free-dim offsets into 7 pre-shifted padded-image variants (host-built).
Each ring's taps form a regular (dy x dx-step) lattice, so all 49 tap
multiplies collapse into 10 DVE ops via multi-dim overlapping views of the
shifted-image tile. Ring sums are pairwise tensor_add trees (tensor_tensor
streams 1 output/cycle using both read ports; strided tensor_reduce only
manages ~0.6/cycle). |k| runs on ScalarE, which has its own SBUF port.
GpSimd is left idle on purpose: its SBUF port is DVE's second read port, so
any concurrent POOL op halves DVE throughput.
"""

import sys

for _p in ("/opt/trn_rl_repo", "/root/.axon_site"):
    if _p not in sys.path:
        sys.path.insert(0, _p)

import numpy as np
from contextlib import ExitStack

import concourse.bass as bass
import concourse.tile as tile
from concourse import bacc, mybir
from concourse.bass_utils import run_bass_kernel_spmd

H, W = 256, 320
BS = 4
KK = 49
HALF_ROWS = 128
PAD_W = W + 6  # 326

def _ring_ids() -> np.ndarray:
    ring = np.zeros(KK, dtype=np.int32)
    for j in range(KK):
        dy, dx = divmod(j, 7)
        ring[j] = max(abs(dy - 3), abs(dx - 3))
    return ring

_RING = _ring_ids()
RING_TAPS = [np.where(_RING == r)[0].tolist() for r in range(4)]  # 1,8,16,24
RING_ORDER = np.concatenate([np.asarray(t) for t in RING_TAPS]).astype(np.int64)

# plane ranges of each ring inside the [128, 49, 320] ring-ordered k tile
RING_OFF = [0, 1, 9, 25, 49]

# tap-multiply op shapes per ring r >= 1: (plane_off, n_planes, img_dims, img_off)
# img_dims are extra AP dims [stride_elems, num] prepended to [1, W];
# img_off is the element offset into the [7, 326] shifted-image block.
# Ring r>=1 taps in j-order: top row (2r+1 taps), middle 2r-1 dy-rows with
# dx in {3-r, 3+r}, bottom row (2r+1 taps).
def _mul_ops(r):
    n = 2 * r + 1
    lo = 3 - r
    ops = []
    ops.append((0, n, [[1, n]], lo * PAD_W + lo))
    ops.append((n, 2 * (n - 2), [[PAD_W, n - 2], [2 * r, 2]], (lo + 1) * PAD_W + lo))
    ops.append((n + 2 * (n - 2), n, [[1, n]], (lo + n - 1) * PAD_W + lo))
    return ops

_NC = None
LAST_RESULTS = None


def _tree_sum(eng, kt, o, sz, dst=None):
    """Pairwise in-place sum of planes [o, o+sz) of kt into plane o.

    sz must be a power of two. If dst is given, the final 2->1 level
    writes there instead.
    """
    assert sz & (sz - 1) == 0
    cur = sz
    while cur > 1:
        h = cur // 2
        out = dst if (dst is not None and cur == 2) else kt[:, o:o + h, :]
        eng.tensor_add(out, kt[:, o:o + h, :], kt[:, o + h:o + 2 * h, :])
        cur = h


def _build_program():
    f32 = mybir.dt.float32
    nc = bacc.Bacc("TRN2", target_bir_lowering=False, debug=False, num_devices=8)
    k_d = nc.dram_tensor("k", [HALF_ROWS, KK, W], f32, kind="ExternalInput").ap()
    img7_d = nc.dram_tensor("img7", [HALF_ROWS, 7, PAD_W], f32, kind="ExternalInput").ap()
    in0_d = nc.dram_tensor("in0", [HALF_ROWS, W], f32, kind="ExternalInput").ap()
    att_d = nc.dram_tensor("att", [HALF_ROWS, 4, W], f32, kind="ExternalInput").ap()
    out_d = nc.dram_tensor("out", [HALF_ROWS, W], f32, kind="ExternalOutput").ap()

    with tile.TileContext(nc) as tc, ExitStack() as ctx:
        pool = ctx.enter_context(tc.tile_pool(name="main", bufs=1))

        kt = pool.tile([HALF_ROWS, KK, W], f32, name="ktile")
        ka = pool.tile([HALF_ROWS, KK - 1, W], f32, name="katile")  # |k|, rings 1-3
        in0_t = pool.tile([HALF_ROWS, W], f32)
        img7_t = pool.tile([HALF_ROWS, 7, PAD_W], f32)
        att_t = pool.tile([HALF_ROWS, 4, W], f32)
        res_b = pool.tile([HALF_ROWS, 4, W], f32)
        res_a = pool.tile([HALF_ROWS, 4, W], f32)

        # DMA order: what unblocks compute soonest goes first
        nc.sync.dma_start(in0_t[:], in0_d[:])
        nc.sync.dma_start(kt[:, 0:9, :], k_d[:, 0:9, :])    # center + ring1
        nc.sync.dma_start(img7_t[:], img7_d[:])
        for o in range(9, KK, 8):                            # rings 2-3, 8-plane chunks
            nc.sync.dma_start(kt[:, o:o + 8, :], k_d[:, o:o + 8, :])
        nc.sync.dma_start(att_t[:], att_d[:])

        img7_ap = img7_t[:]
        part_dim = img7_ap.ap[0]  # [stride, 128]

        # |k| on ScalarE (reads k slabs before the in-place multiplies)
        nc.scalar.activation(res_a[:, 0, :], kt[:, 0, :],
                             mybir.ActivationFunctionType.Abs)
        for o in range(1, KK, 8):
            nc.scalar.activation(ka[:, o - 1:o + 7, :], kt[:, o:o + 8, :],
                                 mybir.ActivationFunctionType.Abs)

        # center tap: B_0 = k_c * input0 (A_0 already in res_a)
        nc.vector.tensor_mul(res_b[:, 0, :], kt[:, 0, :], in0_t[:])

        # k_j *= patches_j, 3 fused ops per ring
        for r in (1, 2, 3):
            for (rel, n_pl, img_dims, img_off) in _mul_ops(r):
                o = RING_OFF[r] + rel
                src = bass.AP(img7_ap.tensor, img7_ap.offset + img_off,
                              [part_dim] + img_dims + [[1, W]])
                nc.vector.tensor_mul(kt[:, o:o + n_pl, :], kt[:, o:o + n_pl, :], src)

        # ring sums: pairwise trees (B in kt, A in ka at offset-1 planes)
        _tree_sum(nc.vector, kt, 1, 8, dst=res_b[:, 1, :])           # ring1
        _tree_sum(nc.vector, ka, 0, 8, dst=res_a[:, 1, :])
        _tree_sum(nc.vector, kt, 9, 16, dst=res_b[:, 2, :])          # ring2
        _tree_sum(nc.vector, ka, 8, 16, dst=res_a[:, 2, :])
        # ring3: fold 3 chunks of 8, then tree the first chunk
        nc.vector.tensor_add(kt[:, 25:33, :], kt[:, 25:33, :], kt[:, 33:41, :])
        nc.vector.tensor_add(kt[:, 25:33, :], kt[:, 25:33, :], kt[:, 41:49, :])
        _tree_sum(nc.vector, kt, 25, 8, dst=res_b[:, 3, :])
        nc.vector.tensor_add(ka[:, 24:32, :], ka[:, 24:32, :], ka[:, 32:40, :])
        nc.vector.tensor_add(ka[:, 24:32, :], ka[:, 24:32, :], ka[:, 40:48, :])
        _tree_sum(nc.vector, ka, 24, 8, dst=res_a[:, 3, :])

        # combine: numer = sum_r att_r*B_r, denom = sum_r att_r*A_r
        pn_t = pool.tile([HALF_ROWS, 4, W], f32)
        nc.vector.tensor_mul(pn_t[:], att_t[:], res_b[:])
        pd_t = pool.tile([HALF_ROWS, 4, W], f32)
        nc.vector.tensor_mul(pd_t[:], att_t[:], res_a[:])
        _tree_sum(nc.vector, pn_t, 0, 4)
        _tree_sum(nc.vector, pd_t, 0, 4)

        rden_t = pool.tile([HALF_ROWS, W], f32)
        nc.vector.reciprocal(rden_t[:], pd_t[:, 0, :])
        out_t = pool.tile([HALF_ROWS, W], f32)
        nc.vector.tensor_mul(out_t[:], pn_t[:, 0, :], rden_t[:])
        nc.sync.dma_start(out_d[:], out_t[:])

    nc.compile()
    return nc


def _get_program():
    global _NC
    if _NC is None:
        _NC = _build_program()
    return _NC


def kernel(**inputs) -> np.ndarray:
    k = np.asarray(inputs["kernel"], dtype=np.float32)      # [4, 49, 81920]
    img = np.asarray(inputs["input"], dtype=np.float32)     # [4, 1, 256, 320]
    in0 = np.asarray(inputs["input0"], dtype=np.float32)    # [4, 1, 256, 320]
    att = np.asarray(inputs["attention"], dtype=np.float32) # [4, 6, 4, 81920]
    ii = int(np.asarray(inputs["i"]))

    nc = _get_program()

    in_maps = []
    for c in range(8):
        b, half = divmod(c, 2)
        y0 = half * HALF_ROWS
        kc = k[b][RING_ORDER][:, y0 * W:(y0 + HALF_ROWS) * W]
        kc = np.ascontiguousarray(kc.reshape(KK, HALF_ROWS, W).transpose(1, 0, 2))
        pad = np.zeros((HALF_ROWS + 6, PAD_W), np.float32)
        lo, hi = max(0, y0 - 3), min(H, y0 + HALF_ROWS + 3)
        pad[lo - (y0 - 3):hi - (y0 - 3), 3:3 + W] = img[b, 0, lo:hi]
        img7 = np.ascontiguousarray(
            np.stack([pad[t:t + HALF_ROWS] for t in range(7)], axis=1))
        in0c = np.ascontiguousarray(in0[b, 0, y0:y0 + HALF_ROWS])
        attc = att[b, ii][:, y0 * W:(y0 + HALF_ROWS) * W]
        attc = np.ascontiguousarray(attc.reshape(4, HALF_ROWS, W).transpose(1, 0, 2))
        in_maps.append({"k": kc, "img7": img7, "in0": in0c, "att": attc})

    res = run_bass_kernel_spmd(nc, in_maps, list(range(8)))
    global LAST_RESULTS
    LAST_RESULTS = res

    out = np.empty((BS, 1, H, W), np.float32)
    for c in range(8):
        b, half = divmod(c, 2)
        out[b, 0, half * HALF_ROWS:(half + 1) * HALF_ROWS] = res.results[c]["out"]
    return out


# revision 10
# speedup vs baseline: 1.3393x; 1.0436x over previous
"""Trainium2 Bass kernel for DYSPN-style dynamic local filtering.

Computation (per batch b, pixel p):
    patches[j,p] = 7x7 im2col of `input` (zero pad 3), center tap replaced by input0
    scale[j,p]   = attention[b, i, ring(j), p]      (ring in {0..3}, scale >= 0)
    w            = kernel * scale;  w /= sum_j |w|
    out[p]       = sum_j patches[j,p] * w[j,p]

Since scale >= 0 and constant within a ring (ring = Chebyshev distance from
the center tap):
    out = (sum_r att_r * B_r) / (sum_r att_r * A_r)
    B_r = sum_{j in ring r} patches_j * k_j,   A_r = sum_{j in ring r} |k_j|

Sharding: 8 cores = 4 batches x 2 half-images (128 rows each). Per core the
output plane is [128 rows (partitions), 320 cols (free)]; tap shifts become
free-dim offsets into 7 pre-shifted padded-image variants (host-built).
Each ring's taps form a regular (dy x dx-step) lattice, so all 49 tap
multiplies collapse into 10 DVE ops via multi-dim overlapping views of the
shifted-image tile. Ring sums are pairwise tensor_add trees (tensor_tensor
streams 1 output/cycle using both read ports; strided tensor_reduce only
manages ~0.6/cycle). |k| runs on ScalarE, which has its own SBUF port; the
|k| planes live 48 planes above the k planes in one tile so each B-tree op
also carries the matching A-tree level as a second AP dim. GpSimd is left
idle on purpose: its SBUF port is DVE's second read port, so any concurrent
POOL op halves DVE throughput.
"""

import sys

for _p in ("/opt/trn_rl_repo", "/root/.axon_site"):
    if _p not in sys.path:
        sys.path.insert(0, _p)

import numpy as np
from contextlib import ExitStack

import concourse.bass as bass
import concourse.tile as tile
from concourse import bacc, mybir
from concourse.bass_utils import run_bass_kernel_spmd

H, W = 256, 320
BS = 4
KK = 49
HALF_ROWS = 128
PAD_W = W + 6  # 326
APLANE = 48  # |k| plane j lives at kall plane j + 48 (j = 1..48)

def _ring_ids() -> np.ndarray:
    ring = np.zeros(KK, dtype=np.int32)
    for j in range(KK):
        dy, dx = divmod(j, 7)
        ring[j] = max(abs(dy - 3), abs(dx - 3))
    return ring

_RING = _ring_ids()
RING_TAPS = [np.where(_RING == r)[0].tolist() for r in range(4)]  # 1,8,16,24
RING_ORDER = np.concatenate([np.asarray(t) for t in RING_TAPS]).astype(np.int64)

# plane ranges of each ring inside the [128, 49, 320] ring-ordered k region
RING_OFF = [0, 1, 9, 25, 49]

def _mul_ops(r):
    """Tap-multiply op shapes for ring r>=1: (rel_plane, n_planes, img_dims, img_off).

    Ring taps in j-order: top row (2r+1), middle 2r-1 rows with dx in
    {3-r, 3+r}, bottom row (2r+1). img_dims are AP dims [stride, num]
    prepended to [1, W]; img_off indexes the [7, 326] shifted-image block.
    """
    n = 2 * r + 1
    lo = 3 - r
    return [
        (0, n, [[1, n]], lo * PAD_W + lo),
        (n, 2 * (n - 2), [[PAD_W, n - 2], [2 * r, 2]], (lo + 1) * PAD_W + lo),
        (n + 2 * (n - 2), n, [[1, n]], (lo + n - 1) * PAD_W + lo),
    ]

_NC = None
LAST_RESULTS = None


def _build_program():
    f32 = mybir.dt.float32
    nc = bacc.Bacc("TRN2", target_bir_lowering=False, debug=False, num_devices=8)
    k_d = nc.dram_tensor("k", [HALF_ROWS, KK, W], f32, kind="ExternalInput").ap()
    img7_d = nc.dram_tensor("img7", [HALF_ROWS, 7, PAD_W], f32, kind="ExternalInput").ap()
    in0_d = nc.dram_tensor("in0", [HALF_ROWS, W], f32, kind="ExternalInput").ap()
    att_d = nc.dram_tensor("att", [HALF_ROWS, 8, W], f32, kind="ExternalInput").ap()
    out_d = nc.dram_tensor("out", [HALF_ROWS, W], f32, kind="ExternalOutput").ap()

    with tile.TileContext(nc) as tc, ExitStack() as ctx:
        pool = ctx.enter_context(tc.tile_pool(name="main", bufs=1))

        # planes 0:49 = ring-ordered k (in-place becomes patches*k);
        # planes 49:97 = |k| for taps 1..48
        kall = pool.tile([HALF_ROWS, 97, W], f32, name="kall")
        img7_t = pool.tile([HALF_ROWS, 7, PAD_W], f32)
        in0_t = pool.tile([HALF_ROWS, W], f32)
        att_t = pool.tile([HALF_ROWS, 8, W], f32)  # att duplicated for B|A paths
        # planes 0:4 = B_r, planes 4:8 = A_r
        res = pool.tile([HALF_ROWS, 8, W], f32)

        kall_ap = kall[:]
        kpart = kall_ap.ap[0]
        img7_ap = img7_t[:]
        ipart = img7_ap.ap[0]

        def kap(plane, dims):
            return bass.AP(kall_ap.tensor, kall_ap.offset + plane * W,
                           [kpart] + dims)

        def iap(off, dims):
            return bass.AP(img7_ap.tensor, img7_ap.offset + off,
                           [ipart] + dims + [[1, W]])

        # ---- DMAs, ordered by when compute needs the data
        nc.sync.dma_start(kall[:, 1:9, :], k_d[:, 1:9, :])          # ring1
        for t in (2, 3, 4):                                         # img rows for ring1
            nc.sync.dma_start(img7_t[:, t, :], img7_d[:, t, :])
        nc.sync.dma_start(in0_t[:], in0_d[:])
        nc.sync.dma_start(kall[:, 0:1, :], k_d[:, 0:1, :])          # center
        nc.sync.dma_start(kall[:, 9:17, :], k_d[:, 9:17, :])        # ring2
        nc.sync.dma_start(kall[:, 17:25, :], k_d[:, 17:25, :])
        for t in (1, 5):
            nc.sync.dma_start(img7_t[:, t, :], img7_d[:, t, :])
        nc.sync.dma_start(kall[:, 25:33, :], k_d[:, 25:33, :])      # ring3
        for t in (0, 6):
            nc.sync.dma_start(img7_t[:, t, :], img7_d[:, t, :])
        nc.sync.dma_start(kall[:, 33:41, :], k_d[:, 33:41, :])
        nc.sync.dma_start(kall[:, 41:49, :], k_d[:, 41:49, :])
        nc.sync.dma_start(att_t[:], att_d[:])

        # ---- |k| on ScalarE (reads k slabs before the in-place multiplies)
        Abs = mybir.ActivationFunctionType.Abs
        nc.scalar.activation(kall[:, 49:53, :], kall[:, 1:5, :], Abs)  # ring1 split
        nc.scalar.activation(kall[:, 53:57, :], kall[:, 5:9, :], Abs)  # for latency
        nc.scalar.activation(res[:, 4, :], kall[:, 0, :], Abs)         # A_0
        for o in range(9, KK, 8):
            nc.scalar.activation(kall[:, o + APLANE:o + 8 + APLANE, :],
                                 kall[:, o:o + 8, :], Abs)

        # ---- patches*k multiplies (3 fused ops per ring + center)
        for r in (1, 2, 3):
            for (rel, n_pl, img_dims, img_off) in _mul_ops(r):
                o = RING_OFF[r] + rel
                nc.vector.tensor_mul(kall[:, o:o + n_pl, :],
                                     kall[:, o:o + n_pl, :],
                                     iap(img_off, img_dims))
        nc.vector.tensor_mul(res[:, 0, :], kall[:, 0, :], in0_t[:])   # B_0

        # ---- ring sums: each op handles the B level and the A level (48
        # planes up) through a paired leading AP dim
        def paired_fold(base, h, delta):
            """kall[{base, base+48}][0:h] += kall[{base+delta, ...}][0:h]"""
            dims = [[APLANE * W, 2], [W, h], [1, W]]
            nc.vector.tensor_add(kap(base, dims), kap(base, dims),
                                 kap(base + delta, dims))

        def paired_tree(base, sz, r):
            cur = sz
            while cur > 2:
                paired_fold(base, cur // 2, cur // 2)
                cur //= 2
            dims = [[APLANE * W, 2], [1, W]]
            rdims = [[4 * W, 2], [1, W]]
            nc.vector.tensor_add(
                bass.AP(res[:].tensor, res[:].offset + r * W, [res[:].ap[0]] + rdims),
                kap(base, dims), kap(base + 1, dims))

        paired_tree(1, 8, 1)                     # ring1
        paired_tree(9, 16, 2)                    # ring2
        paired_fold(25, 8, 8)                    # ring3: fold chunks
        paired_fold(25, 8, 16)
        paired_tree(25, 8, 3)

        # ---- combine + divide
        pnd = pool.tile([HALF_ROWS, 8, W], f32)
        nc.vector.tensor_mul(pnd[:], att_t[:], res[:])
        pnd_ap = pnd[:]
        ppart = pnd_ap.ap[0]

        def pap(plane, dims):
            return bass.AP(pnd_ap.tensor, pnd_ap.offset + plane * W,
                           [ppart] + dims)

        d2 = [[4 * W, 2], [W, 2], [1, W]]
        nc.vector.tensor_add(pap(0, d2), pap(0, d2), pap(2, d2))
        d1 = [[4 * W, 2], [1, W]]
        nc.vector.tensor_add(pap(0, d1), pap(0, d1), pap(1, d1))

        rden_t = pool.tile([HALF_ROWS, W], f32)
        nc.vector.reciprocal(rden_t[:], pnd[:, 4, :])
        out_t = pool.tile([HALF_ROWS, W], f32)
        nc.vector.tensor_mul(out_t[:], pnd[:, 0, :], rden_t[:])
        nc.sync.dma_start(out_d[:], out_t[:])

    nc.compile()
    return nc


def _get_program():
    global _NC
    if _NC is None:
        _NC = _build_program()
    return _NC


def kernel(**inputs) -> np.ndarray:
    k = np.asarray(inputs["kernel"], dtype=np.float32)      # [4, 49, 81920]
    img = np.asarray(inputs["input"], dtype=np.float32)     # [4, 1, 256, 320]
    in0 = np.asarray(inputs["input0"], dtype=np.float32)    # [4, 1, 256, 320]
    att = np.asarray(inputs["attention"], dtype=np.float32) # [4, 6, 4, 81920]
    ii = int(np.asarray(inputs["i"]))

    nc = _get_program()

    in_maps = []
    for c in range(8):
        b, half = divmod(c, 2)
        y0 = half * HALF_ROWS
        kc = k[b][RING_ORDER][:, y0 * W:(y0 + HALF_ROWS) * W]
        kc = np.ascontiguousarray(kc.reshape(KK, HALF_ROWS, W).transpose(1, 0, 2))
        pad = np.zeros((HALF_ROWS + 6, PAD_W), np.float32)
        lo, hi = max(0, y0 - 3), min(H, y0 + HALF_ROWS + 3)
        pad[lo - (y0 - 3):hi - (y0 - 3), 3:3 + W] = img[b, 0, lo:hi]
        img7 = np.ascontiguousarray(
            np.stack([pad[t:t + HALF_ROWS] for t in range(7)], axis=1))
        in0c = np.ascontiguousarray(in0[b, 0, y0:y0 + HALF_ROWS])
        attc = att[b, ii][:, y0 * W:(y0 + HALF_ROWS) * W]
        attc = attc.reshape(4, HALF_ROWS, W).transpose(1, 0, 2)
        att2 = np.ascontiguousarray(np.concatenate([attc, attc], axis=1))
        in_maps.append({"k": kc, "img7": img7, "in0": in0c, "att": att2})

    res = run_bass_kernel_spmd(nc, in_maps, list(range(8)))
    global LAST_RESULTS
    LAST_RESULTS = res

    out = np.empty((BS, 1, H, W), np.float32)
    for c in range(8):
        b, half = divmod(c, 2)
        out[b, 0, half * HALF_ROWS:(half + 1) * HALF_ROWS] = res.results[c]["out"]
    return out


# revision 12
# speedup vs baseline: 1.3554x; 1.0120x over previous
"""Trainium2 Bass kernel for DYSPN-style dynamic local filtering.

Computation (per batch b, pixel p):
    patches[j,p] = 7x7 im2col of `input` (zero pad 3), center tap replaced by input0
    scale[j,p]   = attention[b, i, ring(j), p]      (ring in {0..3}, scale >= 0)
    w            = kernel * scale;  w /= sum_j |w|
    out[p]       = sum_j patches[j,p] * w[j,p]

Since scale >= 0 and constant within a ring (ring = Chebyshev distance from
the center tap):
    out = (sum_r att_r * B_r) / (sum_r att_r * A_r)
    B_r = sum_{j in ring r} patches_j * k_j,   A_r = sum_{j in ring r} |k_j|

Sharding: 8 cores = 4 batches x 2 half-images (128 rows each). Per core the
output plane is [128 rows (partitions), 320 cols (free)]; tap shifts become
free-dim offsets into 7 pre-shifted padded-image variants (host-built).
Each ring's taps form a regular (dy x dx-step) lattice, so all 49 tap
multiplies collapse into 10 DVE ops via multi-dim overlapping views of the
shifted-image tile. Ring sums are pairwise tensor_add trees (tensor_tensor
streams 1 output/cycle using both read ports; strided tensor_reduce only
manages ~0.6/cycle). |k| runs on ScalarE, which has its own SBUF port; the
|k| planes live 48 planes above the k planes in one tile so each B-tree op
also carries the matching A-tree level as a second AP dim. GpSimd is left
idle on purpose: its SBUF port is DVE's second read port, so any concurrent
POOL op halves DVE throughput.
"""

import sys

for _p in ("/opt/trn_rl_repo", "/root/.axon_site"):
    if _p not in sys.path:
        sys.path.insert(0, _p)

import numpy as np
from contextlib import ExitStack

import concourse.bass as bass
import concourse.tile as tile
from concourse import bacc, mybir
from concourse.bass_utils import run_bass_kernel_spmd

H, W = 256, 320
BS = 4
KK = 49
HALF_ROWS = 128
PAD_W = W + 6  # 326
APLANE = 48  # |k| plane j lives at kall plane j + 48 (j = 1..48)

def _ring_ids() -> np.ndarray:
    ring = np.zeros(KK, dtype=np.int32)
    for j in range(KK):
        dy, dx = divmod(j, 7)
        ring[j] = max(abs(dy - 3), abs(dx - 3))
    return ring

_RING = _ring_ids()
RING_TAPS = [np.where(_RING == r)[0].tolist() for r in range(4)]  # 1,8,16,24
RING_ORDER = np.concatenate([np.asarray(t) for t in RING_TAPS]).astype(np.int64)

# plane ranges of each ring inside the [128, 49, 320] ring-ordered k region
RING_OFF = [0, 1, 9, 25, 49]

def _mul_ops(r):
    """Tap-multiply op shapes for ring r>=1: (rel_plane, n_planes, img_dims, img_off).

    Ring taps in j-order: top row (2r+1), middle 2r-1 rows with dx in
    {3-r, 3+r}, bottom row (2r+1). img_dims are AP dims [stride, num]
    prepended to [1, W]; img_off indexes the [7, 326] shifted-image block.
    """
    n = 2 * r + 1
    lo = 3 - r
    return [
        (0, n, [[1, n]], lo * PAD_W + lo),
        (n, 2 * (n - 2), [[PAD_W, n - 2], [2 * r, 2]], (lo + 1) * PAD_W + lo),
        (n + 2 * (n - 2), n, [[1, n]], (lo + n - 1) * PAD_W + lo),
    ]

_NC = None
LAST_RESULTS = None


def _build_program():
    f32 = mybir.dt.float32
    nc = bacc.Bacc("TRN2", target_bir_lowering=False, debug=False, num_devices=8)
    k_d = nc.dram_tensor("k", [HALF_ROWS, KK, W], f32, kind="ExternalInput").ap()
    img7_d = nc.dram_tensor("img7", [HALF_ROWS, 7, PAD_W], f32, kind="ExternalInput").ap()
    in0_d = nc.dram_tensor("in0", [HALF_ROWS, W], f32, kind="ExternalInput").ap()
    att_d = nc.dram_tensor("att", [HALF_ROWS, 8, W], f32, kind="ExternalInput").ap()
    out_d = nc.dram_tensor("out", [HALF_ROWS, W], f32, kind="ExternalOutput").ap()

    with tile.TileContext(nc) as tc, ExitStack() as ctx:
        pool = ctx.enter_context(tc.tile_pool(name="main", bufs=1))

        # planes 0:49 = ring-ordered k (in-place becomes patches*k);
        # planes 49:97 = |k| for taps 1..48
        kall = pool.tile([HALF_ROWS, 97, W], f32, name="kall")
        img7_t = pool.tile([HALF_ROWS, 7, PAD_W], f32)
        in0_t = pool.tile([HALF_ROWS, W], f32)
        att_t = pool.tile([HALF_ROWS, 8, W], f32)  # att duplicated for B|A paths
        # planes 0:4 = B_r, planes 4:8 = A_r
        res = pool.tile([HALF_ROWS, 8, W], f32)

        kall_ap = kall[:]
        kpart = kall_ap.ap[0]
        img7_ap = img7_t[:]
        ipart = img7_ap.ap[0]

        def kap(plane, dims):
            return bass.AP(kall_ap.tensor, kall_ap.offset + plane * W,
                           [kpart] + dims)

        def iap(off, dims):
            return bass.AP(img7_ap.tensor, img7_ap.offset + off,
                           [ipart] + dims + [[1, W]])

        # ---- DMAs, ordered by when compute needs the data
        nc.sync.dma_start(kall[:, 1:5, :], k_d[:, 1:5, :])          # ring1
        nc.sync.dma_start(img7_t[:, 2, :], img7_d[:, 2, :])
        nc.sync.dma_start(kall[:, 5:9, :], k_d[:, 5:9, :])
        for t in (3, 4):                                            # img rows for ring1
            nc.sync.dma_start(img7_t[:, t, :], img7_d[:, t, :])
        nc.sync.dma_start(in0_t[:], in0_d[:])
        nc.sync.dma_start(kall[:, 0:1, :], k_d[:, 0:1, :])          # center
        nc.sync.dma_start(kall[:, 9:17, :], k_d[:, 9:17, :])        # ring2
        nc.sync.dma_start(kall[:, 17:25, :], k_d[:, 17:25, :])
        for t in (1, 5):
            nc.sync.dma_start(img7_t[:, t, :], img7_d[:, t, :])
        nc.sync.dma_start(kall[:, 25:33, :], k_d[:, 25:33, :])      # ring3
        for t in (0, 6):
            nc.sync.dma_start(img7_t[:, t, :], img7_d[:, t, :])
        nc.sync.dma_start(kall[:, 33:41, :], k_d[:, 33:41, :])
        nc.sync.dma_start(kall[:, 41:49, :], k_d[:, 41:49, :])
        nc.sync.dma_start(att_t[:], att_d[:])

        # ---- |k| on ScalarE (reads k slabs before the in-place multiplies)
        Abs = mybir.ActivationFunctionType.Abs
        nc.scalar.activation(kall[:, 49:53, :], kall[:, 1:5, :], Abs)  # ring1 split
        nc.scalar.activation(kall[:, 53:57, :], kall[:, 5:9, :], Abs)  # for latency
        nc.scalar.activation(res[:, 4, :], kall[:, 0, :], Abs)         # A_0
        for o in range(9, KK, 8):
            nc.scalar.activation(kall[:, o + APLANE:o + 8 + APLANE, :],
                                 kall[:, o:o + 8, :], Abs)

        # ---- patches*k multiplies (3 fused ops per ring + center)
        for r in (1, 2, 3):
            for (rel, n_pl, img_dims, img_off) in _mul_ops(r):
                o = RING_OFF[r] + rel
                nc.vector.tensor_mul(kall[:, o:o + n_pl, :],
                                     kall[:, o:o + n_pl, :],
                                     iap(img_off, img_dims))
        nc.vector.tensor_mul(res[:, 0, :], kall[:, 0, :], in0_t[:])   # B_0

        # ---- ring sums: each op handles the B level and the A level (48
        # planes up) through a paired leading AP dim
        def paired_fold(base, h, delta):
            """kall[{base, base+48}][0:h] += kall[{base+delta, ...}][0:h]"""
            dims = [[APLANE * W, 2], [W, h], [1, W]]
            nc.vector.tensor_add(kap(base, dims), kap(base, dims),
                                 kap(base + delta, dims))

        def paired_tree(base, sz, r):
            cur = sz
            while cur > 2:
                paired_fold(base, cur // 2, cur // 2)
                cur //= 2
            dims = [[APLANE * W, 2], [1, W]]
            rdims = [[4 * W, 2], [1, W]]
            nc.vector.tensor_add(
                bass.AP(res[:].tensor, res[:].offset + r * W, [res[:].ap[0]] + rdims),
                kap(base, dims), kap(base + 1, dims))

        paired_tree(1, 8, 1)                     # ring1
        paired_tree(9, 16, 2)                    # ring2
        paired_fold(25, 8, 8)                    # ring3: fold chunks
        paired_fold(25, 8, 16)
        paired_tree(25, 8, 3)

        # ---- combine + divide
        pnd = pool.tile([HALF_ROWS, 8, W], f32)
        nc.vector.tensor_mul(pnd[:], att_t[:], res[:])
        pnd_ap = pnd[:]
        ppart = pnd_ap.ap[0]

        def pap(plane, dims):
            return bass.AP(pnd_ap.tensor, pnd_ap.offset + plane * W,
                           [ppart] + dims)

        d2 = [[4 * W, 2], [W, 2], [1, W]]
        nc.vector.tensor_add(pap(0, d2), pap(0, d2), pap(2, d2))
        d1 = [[4 * W, 2], [1, W]]
        nc.vector.tensor_add(pap(0, d1), pap(0, d1), pap(1, d1))

        rden_t = pool.tile([HALF_ROWS, W], f32)
        scr_t = pool.tile([HALF_ROWS, W], f32)
        nc.vector.reciprocal_approx_accurate(rden_t[:], pnd[:, 4, :], scr_t[:])
        out_t = pool.tile([HALF_ROWS, W], f32)
        nc.vector.tensor_mul(out_t[:], pnd[:, 0, :], rden_t[:])
        nc.sync.dma_start(out_d[:], out_t[:])

    nc.compile()
    return nc


def _get_program():
    global _NC
    if _NC is None:
        _NC = _build_program()
    return _NC


def kernel(**inputs) -> np.ndarray:
    k = np.asarray(inputs["kernel"], dtype=np.float32)      # [4, 49, 81920]
    img = np.asarray(inputs["input"], dtype=np.float32)     # [4, 1, 256, 320]
    in0 = np.asarray(inputs["input0"], dtype=np.float32)    # [4, 1, 256, 320]
    att = np.asarray(inputs["attention"], dtype=np.float32) # [4, 6, 4, 81920]
    ii = int(np.asarray(inputs["i"]))

    nc = _get_program()

    in_maps = []
    for c in range(8):
        b, half = divmod(c, 2)
        y0 = half * HALF_ROWS
        kc = k[b][RING_ORDER][:, y0 * W:(y0 + HALF_ROWS) * W]
        kc = np.ascontiguousarray(kc.reshape(KK, HALF_ROWS, W).transpose(1, 0, 2))
        pad = np.zeros((HALF_ROWS + 6, PAD_W), np.float32)
        lo, hi = max(0, y0 - 3), min(H, y0 + HALF_ROWS + 3)
        pad[lo - (y0 - 3):hi - (y0 - 3), 3:3 + W] = img[b, 0, lo:hi]
        img7 = np.ascontiguousarray(
            np.stack([pad[t:t + HALF_ROWS] for t in range(7)], axis=1))
        in0c = np.ascontiguousarray(in0[b, 0, y0:y0 + HALF_ROWS])
        attc = att[b, ii][:, y0 * W:(y0 + HALF_ROWS) * W]
        attc = attc.reshape(4, HALF_ROWS, W).transpose(1, 0, 2)
        att2 = np.ascontiguousarray(np.concatenate([attc, attc], axis=1))
        in_maps.append({"k": kc, "img7": img7, "in0": in0c, "att": att2})

    res = run_bass_kernel_spmd(nc, in_maps, list(range(8)))
    global LAST_RESULTS
    LAST_RESULTS = res

    out = np.empty((BS, 1, H, W), np.float32)
    for c in range(8):
        b, half = divmod(c, 2)
        out[b, 0, half * HALF_ROWS:(half + 1) * HALF_ROWS] = res.results[c]["out"]
    return out
